# revision 25
# baseline (speedup 1.0000x reference)
"""Distributed Trainium2 kernel for nn_Attention_10857677324470.

Sharding: 8 NeuronCores = batch item b (4) x head-half g (2).
Each core computes, for its (item, head-group-of-4):
  qkv 1x1 conv (768 of 1536 out-channels) -> depthwise 3x3 -> linear
  attention for 4 heads -> crpe refine -> partial output channels.
Cross-core communication is a pairwise ppermute exchange of the 256-ch
attention output between the two cores sharing one batch item; each core
then computes its 256 proj output rows as two 256-col matmuls (own half +
received half), which avoids materializing the gathered [512, N] tensor.

crpe (per-head 64ch -> 1ch windowed conv) is NOT run as a grouped conv
(XLA-Neuron lowers that terribly). Instead: one tap-matmul
z[h,t,n] = sum_c w[h,c,t] v[h,c,n] on the tensor engine, then a 49-term
shifted spatial sum of z (cheap elementwise adds).

Channel-norms are computed as self-contractions (einsum -> tensor engine)
instead of elementwise square + reduce on the vector engine.

All weights are packed host-side into a single flat f32 array (fewer pmap
args = less per-dispatch marshaling over the axon tunnel).
"""

import numpy as np
import jax
import jax.numpy as jnp
from functools import partial

P = 2
HEADS = 8
HG = 2                 # head groups (shards per item)
HPG = HEADS // HG      # heads per group = 4
FF = 4
EPS = 1e-8
WINDOWS = [3, 5, 7]
HEAD_SPLITS = [2, 3, 3]
C = 512
CH = C // HEADS        # 64
H = W = 128
N = H * W
NCORES = 8
KW = 7                 # crpe window padded to 7x7
KT = KW * KW           # 49 taps
PAD = KW // 2

_AXIS_GROUPS = [[0, 1], [2, 3], [4, 5], [6, 7]]
_PPERM = [(0, 1), (1, 0), (2, 3), (3, 2), (4, 5), (5, 4), (6, 7), (7, 6)]

# flat weight-pack offsets (f32 counts)
_SZ = {
    "qkv_w": 768 * 512,
    "dw_w": 768 * 9,
    "proj_own": 512 * 256,
    "proj_other": 512 * 256,
    "crpe_wz": HPG * CH * KT,
    "crpe_b": HPG,
    "temp": HPG,
    "sg": HPG,
}
_OFF = {}
_o = 0
for _k, _s in _SZ.items():
    _OFF[_k] = _o
    _o += _s
_WPACK_LEN = _o


def _head_window(h):
    # head -> (crpe index, local index within that crpe's split)
    if h < 2:
        return 0, h
    if h < 5:
        return 1, h - 2
    return 2, h - 5


def _dw3x3(t, w):
    """Depthwise 3x3 as 9 shifted multiply-adds.

    t: [C', H, W] (bf16 reads, f32 accumulation via the f32 weights),
    w: [C', 3, 3] f32.
    """
    tp = jnp.pad(t.astype(jnp.bfloat16), ((0, 0), (1, 1), (1, 1)))
    out = None
    for dy in range(3):
        for dx in range(3):
            term = w[:, dy, dx][:, None, None] * \
                jax.lax.dynamic_slice(tp, (0, dy, dx), tp.shape[:1] + (H, W))
            out = term if out is None else out + term
    return out


@partial(jax.pmap, axis_name="x")
def _device_fn(x, wpack):
    """Per-core computation.

    x:     [512, 128, 128] bf16 input image for this core's batch item
    wpack: [_WPACK_LEN] f32 all weights, packed (see _OFF/_SZ)
    """
    f32 = jnp.float32
    bf16 = jnp.bfloat16

    def seg(name, shape):
        return jax.lax.dynamic_slice(wpack, (_OFF[name],), (_SZ[name],)).reshape(shape)

    qkv_w = seg("qkv_w", (768, 512))
    dw_w = seg("dw_w", (768, 3, 3))
    proj_own = seg("proj_own", (512, 256))
    proj_other = seg("proj_other", (512, 256))
    crpe_wz = seg("crpe_wz", (HPG, CH, KT)).astype(bf16)
    crpe_b = seg("crpe_b", (HPG,))
    temp = seg("temp", (HPG,))[:, None, None]
    sg = seg("sg", (HPG,))[:, None, None]

    # qkv 1x1 conv as bf16 matmul with fp32 accumulation (x arrives bf16)
    xf = x.reshape(C, N)
    qkv = jax.lax.dot(qkv_w.astype(bf16), xf,
                      preferred_element_type=f32)       # [768, N]
    qkv = _dw3x3(qkv.reshape(768, H, W), dw_w).reshape(768, N)

    q = qkv[0:256].reshape(HPG, CH, N)
    k = qkv[256:512].reshape(HPG, CH, N)
    v = qkv[512:768].reshape(HPG, CH, N)

    # channel-norms as tensor-engine self-contractions
    qb = q.astype(bf16)
    kb = k.astype(bf16)
    vb = v.astype(bf16)
    qr = jax.nn.relu(qb)
    kr = jax.nn.relu(kb)
    qr2 = qr * qr
    kr2 = kr * kr
    q4 = qr2 * qr2                                      # relu(q)^4, bf16
    k4 = kr2 * kr2

    def sumsq(t):   # [h, c, n] -> [h, 1, n], f32, contraction over c on PE
        return jnp.einsum("hcn,hcn->hn", t, t,
                          preferred_element_type=f32)[:, None, :]

    q1s = 1.0 / (jnp.sqrt(sumsq(qb)) + EPS)
    k1s = 1.0 / (jnp.sqrt(sumsq(kb)) + EPS)
    q2s = 1.0 / (jnp.sqrt(sumsq(q4)) + EPS)
    k2s = 1.0 / (jnp.sqrt(sumsq(k4)) + EPS)

    q1 = (qb * q1s).astype(bf16)                        # h c n
    k1 = (kb * k1s).astype(bf16)
    q2 = (q4 * q2s).astype(bf16)
    k2 = (k4 * k2s).astype(bf16)

    # crpe via tap-matmul + shifted sum (no grouped conv)
    z = jnp.einsum("hct,hcn->htn", crpe_wz, vb,
                   preferred_element_type=f32)          # [h, 49, N]
    zp = jnp.pad(z.reshape(HPG, KT, H, W),
                 ((0, 0), (0, 0), (PAD, PAD), (PAD, PAD)))
    att = None
    for dy in range(KW):
        for dx in range(KW):
            t = KW * dy + dx
            term = jax.lax.dynamic_slice(
                zp, (0, t, dy, dx), (HPG, 1, H, W))[:, 0]
            att = term if att is None else att + term
    refine = jax.nn.sigmoid(
        att.reshape(HPG, N) + crpe_b[:, None]).reshape(HPG, 1, N)

    # linear attention (channel-major throughout; contraction over pixels)
    attn1 = jax.lax.dot_general(k1, vb, (((2,), (2,)), ((0,), (0,))),
                                preferred_element_type=f32)   # h c d
    attn2 = jax.lax.dot_general(k2, vb, (((2,), (2,)), ((0,), (0,))),
                                preferred_element_type=f32)
    vsum = v.sum(axis=2, keepdims=True)                 # h d 1
    m1 = jax.lax.dot_general(attn1.astype(bf16), q1,
                             (((1,), (1,)), ((0,), (0,))),
                             preferred_element_type=f32)      # h d n
    m2 = jax.lax.dot_general((sg * attn2).astype(bf16), q2,
                             (((1,), (1,)), ((0,), (0,))),
                             preferred_element_type=f32)
    numer = vsum + m1 + m2                              # h d n
    k1sum = k1.astype(f32).sum(axis=-1)                 # h c (f32 accumulate)
    k2sum = k2.astype(f32).sum(axis=-1) * sg[:, :, 0]   # h c, scale folded in
    s1 = jnp.einsum("hcn,hc->hn", q1, k1sum.astype(bf16),
                    preferred_element_type=f32)
    s2 = jnp.einsum("hcn,hc->hn", q2, k2sum.astype(bf16),
                    preferred_element_type=f32)
    denom = f32(N) + s1[:, None, :] + s2[:, None, :] + EPS
    out = ((numer / denom) * temp + refine).reshape(256, N).astype(bf16)

    # N-split proj: each core computes ALL 512 proj rows for its item on
    # its own half of the pixels, so only the half the partner needs is
    # exchanged (4.2 MB instead of 8.4 MB). g = this core's head-group.
    half = N // 2
    g = jax.lax.axis_index("x") % 2
    send = jax.lax.dynamic_slice(out, (0, (1 - g) * half), (256, half))
    recv = jax.lax.ppermute(send, "x", perm=_PPERM)      # partner's 256 ch
    own = jax.lax.dynamic_slice(out, (0, g * half), (256, half))
    o = jax.lax.dot(proj_own.astype(bf16), own, preferred_element_type=f32)
    o = o + jax.lax.dot(proj_other.astype(bf16), recv,
                        preferred_element_type=f32)
    # return bf16: halves the D2H transfer over the slow tunnel; host
    # casts back to f32 (rounding ~0.4%, well within the error budget)
    return o.reshape(512, H // 2, W).astype(bf16)


def _build_args(x, qkv_w, dw_w, proj_w, temperature, scale,
                crpe_w0, crpe_b0, crpe_w1, crpe_b1, crpe_w2, crpe_b2):
    x = np.asarray(x, dtype=np.float32)
    qkv_w = np.asarray(qkv_w, dtype=np.float32).reshape(3072, 512)
    dw_w = np.asarray(dw_w, dtype=np.float32).reshape(3072, 3, 3)
    proj_w = np.asarray(proj_w, dtype=np.float32).reshape(1024, 512)
    temperature = np.asarray(temperature, dtype=np.float32)
    scale = np.asarray(scale, dtype=np.float32)
    crpe_ws = [np.asarray(w, dtype=np.float32) for w in (crpe_w0, crpe_w1, crpe_w2)]
    crpe_bs = [np.asarray(b, dtype=np.float32) for b in (crpe_b0, crpe_b1, crpe_b2)]

    b = x.shape[0]
    B = b // P

    xs, wpacks = [], []
    for core in range(NCORES):
        item = core // HG
        g = core % HG
        go = 1 - g                         # the paired core's head group
        p = item // B                      # path of this batch item
        heads = list(range(g * HPG, (g + 1) * HPG))

        wp = np.zeros((_WPACK_LEN,), dtype=np.float32)

        def put(name, arr):
            wp[_OFF[name]:_OFF[name] + _SZ[name]] = np.asarray(
                arr, dtype=np.float32).ravel()

        # qkv rows for path p: q block then k,v; within each, this group's heads
        base = p * 3 * C
        rows = []
        for sec in range(3):               # q, k, v sections
            lo = base + sec * C + g * HPG * CH
            rows.append(np.arange(lo, lo + HPG * CH))
        rows = np.concatenate(rows)
        put("qkv_w", qkv_w[rows])
        put("dw_w", dw_w[rows])

        # crpe filters as [h, c, tap] with every window zero-centered in 7x7
        cwz = np.zeros((HPG, CH, KT), dtype=np.float32)
        cb = np.zeros((HPG,), dtype=np.float32)
        for j, hh in enumerate(heads):
            wi, li = _head_window(hh)
            hs = HEAD_SPLITS[wi]
            win = WINDOWS[wi]
            pad = (KW - win) // 2
            full = np.zeros((CH, KW, KW), dtype=np.float32)
            full[:, pad:KW - pad, pad:KW - pad] = crpe_ws[wi][p * hs + li]
            cwz[j] = full.reshape(CH, KT)
            cb[j] = crpe_bs[wi][p * hs + li]
        put("crpe_wz", cwz)
        put("crpe_b", cb)

        # full 512 proj rows for this item, cols split by producing core
        pr = proj_w[p * C:(p + 1) * C]                        # [512, 512]
        put("proj_own", pr[:, g * 256:(g + 1) * 256])
        put("proj_other", pr[:, go * 256:(go + 1) * 256])

        put("temp", temperature[p, heads, 0, 0])
        put("sg", 1.0 / (1.0 + np.exp(-scale[p, heads, 0, 0])))

        wpacks.append(wp)
        # ship x as bf16: the device casts to bf16 before the qkv matmul
        # anyway, so this halves the dominant transfer at no numerical cost
        xs.append(x[item].astype(jnp.bfloat16))

    return [np.stack(xs), np.stack(wpacks)], b


def _assemble(outs, b):
    result = np.empty((b, C, H, W), dtype=np.float32)
    for core in range(NCORES):
        item, g = core // HG, core % HG
        result[item, :, g * (H // 2):(g + 1) * (H // 2), :] = \
            np.asarray(outs[core], dtype=np.float32)
    return result


def kernel(**inputs):
    args, b = _build_args(**inputs)
    outs = np.asarray(_device_fn(*args))   # [8, 512, 64, 128]
    return _assemble(outs, b)


# revision 26
# speedup vs baseline: 1.0016x; 1.0016x over previous
"""Distributed Trainium2 kernel for nn_Attention_10857677324470.

Sharding: 8 NeuronCores = batch item b (4) x head-half g (2).
Each core computes, for its (item, head-group-of-4):
  qkv 1x1 conv (768 of 1536 out-channels) -> depthwise 3x3 -> linear
  attention for 4 heads -> crpe refine -> partial output channels.
Cross-core communication is a pairwise ppermute exchange of the 256-ch
attention output between the two cores sharing one batch item; each core
then computes its 256 proj output rows as two 256-col matmuls (own half +
received half), which avoids materializing the gathered [512, N] tensor.

crpe (per-head 64ch -> 1ch windowed conv) is NOT run as a grouped conv
(XLA-Neuron lowers that terribly). Instead: one tap-matmul
z[h,t,n] = sum_c w[h,c,t] v[h,c,n] on the tensor engine, then a 49-term
shifted spatial sum of z (cheap elementwise adds).

Channel-norms are computed as self-contractions (einsum -> tensor engine)
instead of elementwise square + reduce on the vector engine.

All weights are packed host-side into a single flat f32 array (fewer pmap
args = less per-dispatch marshaling over the axon tunnel).
"""

import numpy as np
import jax
import jax.numpy as jnp
from functools import partial

P = 2
HEADS = 8
HG = 2                 # head groups (shards per item)
HPG = HEADS // HG      # heads per group = 4
FF = 4
EPS = 1e-8
WINDOWS = [3, 5, 7]
HEAD_SPLITS = [2, 3, 3]
C = 512
CH = C // HEADS        # 64
H = W = 128
N = H * W
NCORES = 8
KW = 7                 # crpe window padded to 7x7
KT = KW * KW           # 49 taps
PAD = KW // 2

_AXIS_GROUPS = [[0, 1], [2, 3], [4, 5], [6, 7]]
_PPERM = [(0, 1), (1, 0), (2, 3), (3, 2), (4, 5), (5, 4), (6, 7), (7, 6)]

# flat weight-pack offsets (f32 counts)
_SZ = {
    "qkv_w": 768 * 512,
    "dw_w": 768 * 9,
    "proj_own": 256 * 256,
    "proj_other": 256 * 256,
    "crpe_wz": HPG * CH * KT,
    "crpe_b": HPG,
    "temp": HPG,
    "sg": HPG,
}
_OFF = {}
_o = 0
for _k, _s in _SZ.items():
    _OFF[_k] = _o
    _o += _s
_WPACK_LEN = _o


def _head_window(h):
    # head -> (crpe index, local index within that crpe's split)
    if h < 2:
        return 0, h
    if h < 5:
        return 1, h - 2
    return 2, h - 5


def _dw3x3(t, w):
    """Depthwise 3x3 as 9 shifted multiply-adds.

    t: [C', H, W] (bf16 reads, f32 accumulation via the f32 weights),
    w: [C', 3, 3] f32.
    """
    tp = jnp.pad(t.astype(jnp.bfloat16), ((0, 0), (1, 1), (1, 1)))
    out = None
    for dy in range(3):
        for dx in range(3):
            term = w[:, dy, dx][:, None, None] * \
                jax.lax.dynamic_slice(tp, (0, dy, dx), tp.shape[:1] + (H, W))
            out = term if out is None else out + term
    return out


@partial(jax.pmap, axis_name="x")
def _device_fn(x, wpack):
    """Per-core computation.

    x:     [512, 128, 128] bf16 input image for this core's batch item
    wpack: [_WPACK_LEN] f32 all weights, packed (see _OFF/_SZ)
    """
    f32 = jnp.float32
    bf16 = jnp.bfloat16

    def seg(name, shape):
        return jax.lax.dynamic_slice(wpack, (_OFF[name],), (_SZ[name],)).reshape(shape)

    qkv_w = seg("qkv_w", (768, 512))
    dw_w = seg("dw_w", (768, 3, 3))
    proj_own = seg("proj_own", (256, 256))
    proj_other = seg("proj_other", (256, 256))
    crpe_wz = seg("crpe_wz", (HPG, CH, KT)).astype(bf16)
    crpe_b = seg("crpe_b", (HPG,))
    temp = seg("temp", (HPG,))[:, None, None]
    sg = seg("sg", (HPG,))[:, None, None]

    # qkv 1x1 conv as bf16 matmul with fp32 accumulation (x arrives bf16)
    xf = x.reshape(C, N)
    qkv = jax.lax.dot(qkv_w.astype(bf16), xf,
                      preferred_element_type=f32)       # [768, N]
    qkv = _dw3x3(qkv.reshape(768, H, W), dw_w).reshape(768, N)

    q = qkv[0:256].reshape(HPG, CH, N)
    k = qkv[256:512].reshape(HPG, CH, N)
    v = qkv[512:768].reshape(HPG, CH, N)

    # channel-norms as tensor-engine self-contractions
    qb = q.astype(bf16)
    kb = k.astype(bf16)
    vb = v.astype(bf16)
    qr = jax.nn.relu(qb)
    kr = jax.nn.relu(kb)
    qr2 = qr * qr
    kr2 = kr * kr
    q4 = qr2 * qr2                                      # relu(q)^4, bf16
    k4 = kr2 * kr2

    def sumsq(t):   # [h, c, n] -> [h, 1, n], f32, contraction over c on PE
        return jnp.einsum("hcn,hcn->hn", t, t,
                          preferred_element_type=f32)[:, None, :]

    q1s = 1.0 / (jnp.sqrt(sumsq(qb)) + EPS)
    k1s = 1.0 / (jnp.sqrt(sumsq(kb)) + EPS)
    q2s = 1.0 / (jnp.sqrt(sumsq(q4)) + EPS)
    k2s = 1.0 / (jnp.sqrt(sumsq(k4)) + EPS)

    q1 = (qb * q1s).astype(bf16)                        # h c n
    k1 = (kb * k1s).astype(bf16)
    q2 = (q4 * q2s).astype(bf16)
    k2 = (k4 * k2s).astype(bf16)

    # crpe via tap-matmul + shifted sum (no grouped conv)
    z = jnp.einsum("hct,hcn->htn", crpe_wz, vb,
                   preferred_element_type=f32)          # [h, 49, N]
    zp = jnp.pad(z.reshape(HPG, KT, H, W),
                 ((0, 0), (0, 0), (PAD, PAD), (PAD, PAD)))
    att = None
    for dy in range(KW):
        for dx in range(KW):
            t = KW * dy + dx
            term = jax.lax.dynamic_slice(
                zp, (0, t, dy, dx), (HPG, 1, H, W))[:, 0]
            att = term if att is None else att + term
    refine = jax.nn.sigmoid(
        att.reshape(HPG, N) + crpe_b[:, None]).reshape(HPG, 1, N)

    # linear attention (channel-major throughout; contraction over pixels)
    attn1 = jax.lax.dot_general(k1, vb, (((2,), (2,)), ((0,), (0,))),
                                preferred_element_type=f32)   # h c d
    attn2 = jax.lax.dot_general(k2, vb, (((2,), (2,)), ((0,), (0,))),
                                preferred_element_type=f32)
    vsum = v.sum(axis=2, keepdims=True)                 # h d 1
    m1 = jax.lax.dot_general(attn1.astype(bf16), q1,
                             (((1,), (1,)), ((0,), (0,))),
                             preferred_element_type=f32)      # h d n
    m2 = jax.lax.dot_general((sg * attn2).astype(bf16), q2,
                             (((1,), (1,)), ((0,), (0,))),
                             preferred_element_type=f32)
    numer = vsum + m1 + m2                              # h d n
    k1sum = k1.astype(f32).sum(axis=-1)                 # h c (f32 accumulate)
    k2sum = k2.astype(f32).sum(axis=-1) * sg[:, :, 0]   # h c, scale folded in
    s1 = jnp.einsum("hcn,hc->hn", q1, k1sum.astype(bf16),
                    preferred_element_type=f32)
    s2 = jnp.einsum("hcn,hc->hn", q2, k2sum.astype(bf16),
                    preferred_element_type=f32)
    denom = f32(N) + s1[:, None, :] + s2[:, None, :] + EPS
    out = ((numer / denom) * temp + refine).reshape(256, N).astype(bf16)

    # pairwise exchange of the other head-half, split in two so the
    # own-half matmul can overlap the transfer; then proj as three matmuls
    half = N // 2
    other_a = jax.lax.ppermute(out[:, :half], "x", perm=_PPERM)
    other_b = jax.lax.ppermute(out[:, half:], "x", perm=_PPERM)
    o = jax.lax.dot(proj_own.astype(bf16), out, preferred_element_type=f32)
    po = proj_other.astype(bf16)
    oo = jnp.concatenate(
        [jax.lax.dot(po, other_a, preferred_element_type=f32),
         jax.lax.dot(po, other_b, preferred_element_type=f32)], axis=1)
    o = o + oo
    # return bf16: halves the D2H transfer over the slow tunnel; host
    # casts back to f32 (rounding ~0.4%, well within the error budget)
    return o.reshape(256, H, W).astype(bf16)


def _build_args(x, qkv_w, dw_w, proj_w, temperature, scale,
                crpe_w0, crpe_b0, crpe_w1, crpe_b1, crpe_w2, crpe_b2):
    x = np.asarray(x, dtype=np.float32)
    qkv_w = np.asarray(qkv_w, dtype=np.float32).reshape(3072, 512)
    dw_w = np.asarray(dw_w, dtype=np.float32).reshape(3072, 3, 3)
    proj_w = np.asarray(proj_w, dtype=np.float32).reshape(1024, 512)
    temperature = np.asarray(temperature, dtype=np.float32)
    scale = np.asarray(scale, dtype=np.float32)
    crpe_ws = [np.asarray(w, dtype=np.float32) for w in (crpe_w0, crpe_w1, crpe_w2)]
    crpe_bs = [np.asarray(b, dtype=np.float32) for b in (crpe_b0, crpe_b1, crpe_b2)]

    b = x.shape[0]
    B = b // P

    xs, wpacks = [], []
    for core in range(NCORES):
        item = core // HG
        g = core % HG
        go = 1 - g                         # the paired core's head group
        p = item // B                      # path of this batch item
        heads = list(range(g * HPG, (g + 1) * HPG))

        wp = np.zeros((_WPACK_LEN,), dtype=np.float32)

        def put(name, arr):
            wp[_OFF[name]:_OFF[name] + _SZ[name]] = np.asarray(
                arr, dtype=np.float32).ravel()

        # qkv rows for path p: q block then k,v; within each, this group's heads
        base = p * 3 * C
        rows = []
        for sec in range(3):               # q, k, v sections
            lo = base + sec * C + g * HPG * CH
            rows.append(np.arange(lo, lo + HPG * CH))
        rows = np.concatenate(rows)
        put("qkv_w", qkv_w[rows])
        put("dw_w", dw_w[rows])

        # crpe filters as [h, c, tap] with every window zero-centered in 7x7
        cwz = np.zeros((HPG, CH, KT), dtype=np.float32)
        cb = np.zeros((HPG,), dtype=np.float32)
        for j, hh in enumerate(heads):
            wi, li = _head_window(hh)
            hs = HEAD_SPLITS[wi]
            win = WINDOWS[wi]
            pad = (KW - win) // 2
            full = np.zeros((CH, KW, KW), dtype=np.float32)
            full[:, pad:KW - pad, pad:KW - pad] = crpe_ws[wi][p * hs + li]
            cwz[j] = full.reshape(CH, KT)
            cb[j] = crpe_bs[wi][p * hs + li]
        put("crpe_wz", cwz)
        put("crpe_b", cb)

        # proj rows for this core, split by which core holds the input cols
        pr = proj_w[p * C + g * 256: p * C + (g + 1) * 256]   # [256, 512]
        put("proj_own", pr[:, g * 256:(g + 1) * 256])
        put("proj_other", pr[:, go * 256:(go + 1) * 256])

        put("temp", temperature[p, heads, 0, 0])
        put("sg", 1.0 / (1.0 + np.exp(-scale[p, heads, 0, 0])))

        wpacks.append(wp)
        # ship x as bf16: the device casts to bf16 before the qkv matmul
        # anyway, so this halves the dominant transfer at no numerical cost
        xs.append(x[item].astype(jnp.bfloat16))

    return [np.stack(xs), np.stack(wpacks)], b


def _assemble(outs, b):
    result = np.empty((b, C, H, W), dtype=np.float32)
    for core in range(NCORES):
        item, g = core // HG, core % HG
        result[item, g * 256:(g + 1) * 256] = np.asarray(outs[core], dtype=np.float32)
    return result


def kernel(**inputs):
    args, b = _build_args(**inputs)
    outs = np.asarray(_device_fn(*args))   # [8, 256, 128, 128]
    return _assemble(outs, b)
